# revision 6
# baseline (speedup 1.0000x reference)
"""CaMoE block (LayerNorm -> per-expert squared-ReLU FFN with top-1 routing,
confidence-scaled combine, residual) on 8 Trainium2 NeuronCores.

v2 strategy (64-token quanta, slot-granular matmul phases):
  * Host: stable-sort tokens by winning expert; pack into per-core SLOTS
    sized in 64-token quanta, identical slot structure across cores (SPMD).
    All slot sizes are even (multiples of 128 tokens) except possibly the
    last, so every slot boundary is 128-aligned; per-core token count is
    64*ceil(sum_e ceil(cnt_e/64) / 8) -- less padding than 128-tile packing.
  * Device (per core):
      stage A (per 128-token tile): LayerNorm stats + confidence
        sigmoid(rs*(h.wc)+bc) and straight-through scale
        sc = conf/(conf+1e-6) * rs^2 (1/std folded out of h, it commutes
        through relu^2 into the final scale); h = x - mu in bf16;
        hT built by XBAR DMA-transpose (no PE/PSUM involvement).
      per slot: mm1 phase (W1 chunks streamed, pk = W1c^T @ hT over all
        slot tokens at once, 256..448 moving cols -> LDWEIGHTS fully
        hidden), relu^2 on DVE -> kt (whole slot resident in SBUF);
        mm2 phase in two C-halves (ys[block, half] psum is one bank each):
        ys += kt_block^T @ W2chunk, combine ot = ys*sc + x, DMA out.
  * PE warm-up matmuls at program start keep the HAM clock at 2.4 GHz by
    the time the first real matmul issues.
  * DMA submissions split across queues: sync=weight streams,
    scalar(ACT)=hT transposes (ordered after the h they read),
    gpsimd(SWDGE)=x/wcb/bc/out.
  * Host: scatter rows back to original token positions.

gamma/beta are folded into W1/wc on the host (beta==0 fast path assumed,
with a general fallback mirroring the math via an H-bias).
Matmuls in bf16 with fp32 PSUM accumulation.
"""

import math
import os
from contextlib import ExitStack

import numpy as np

import concourse.bass as bass
import concourse.mybir as mybir
import concourse.tile as tile
from concourse.bass_utils import run_bass_kernel_spmd
from concourse.tile import TileContext, ScopedClock

AF = mybir.ActivationFunctionType
OP = mybir.AluOpType
BF16 = mybir.dt.bfloat16
F32 = mybir.dt.float32
NP_BF16 = mybir.dt.np(BF16)

NCORES = 8
TILE = 128
QT = 64                  # token quantum
LN_EPS = 1e-5
WARMUP_MM = 64

# ---------------------------------------------------------------------------
# Walrus workarounds (single-wait encoding), same as v1.
# ---------------------------------------------------------------------------


def _patched_drain_and_barrier(self, tick_clock, wait_clock):
    probe = self.nc.sync.nop(nofuse=True)
    wait_clock.add_sem_waits(probe.ins, ScopedClock({None: tick_clock.global_clock}))
    si = probe.ins.sync_info
    waits = list(si.on_wait) if si is not None and si.on_wait else []
    if len(waits) > 1:
        probe.ins.sync_info = mybir.SyncInfo(on_wait=[waits[0]], on_update=[])
        for w in waits[1:]:
            n = self.nc.sync.nop(nofuse=True)
            n.ins.sync_info = mybir.SyncInfo(on_wait=[w], on_update=[])
    self.nc.sync.drain()
    self.nc.all_engine_barrier()
    assert self.sems is not None
    popped = self.nc._tile_sem_poison_stack.pop()
    assert popped is self._sem_poison
    self.nc.clear_and_free_semaphores(list(self.sems.allocated().values()))
    self.nc.all_engine_barrier()


TileContext._drain_and_barrier = _patched_drain_and_barrier


def _split_excess_waits(nc, max_waits: int = 1):
    for fn in nc.m.functions:
        for bb in fn.blocks:
            insts = list(bb.instructions)
            out = []
            changed = False
            for inst in insts:
                si = inst.sync_info
                waits = list(si.on_wait) if si is not None and si.on_wait else []
                if len(waits) > max_waits:
                    extra = waits[:-max_waits]
                    keep = waits[-max_waits:]
                    for j, w in enumerate(extra):
                        nop = mybir.InstNoOp(
                            name=f"{inst.name}-wsplit{j}", ins=[], outs=[]
                        )
                        nop.engine = inst.engine
                        nop.sync_info = mybir.SyncInfo(on_wait=[w], on_update=[])
                        out.append(nop)
                    inst.sync_info = mybir.SyncInfo(
                        on_wait=keep,
                        on_update=list(si.on_update) if si.on_update else [],
                    )
                    changed = True
                out.append(inst)
            if changed:
                bb.instructions = out


# ---------------------------------------------------------------------------
# Device program
# ---------------------------------------------------------------------------


def _slot_geometry(Tvec, M):
    """Per-slot (tok_off, ntok), LN tiles (global 128-blocks), per-slot
    mm2 blocks [(col_off_in_slot, width, global_tile_idx), ...]."""
    slots = []
    off = 0
    for q in Tvec:
        slots.append((off, q * QT))
        off += q * QT
    assert off == M
    T = (M + TILE - 1) // TILE
    tiles = [(t * TILE, min(TILE, M - t * TILE)) for t in range(T)]
    slot_blocks = []
    for (soff, ntok) in slots:
        blocks = []
        c = 0
        while c < ntok:
            w = min(TILE, ntok - c)
            gt = (soff + c) // TILE
            assert (soff + c) % TILE == 0
            blocks.append((c, w, gt))
            c += w
        slot_blocks.append(blocks)
    # tile -> slot
    tile_slot = []
    for (r0, _rows) in tiles:
        for si, (soff, ntok) in enumerate(slots):
            if soff <= r0 < soff + ntok:
                tile_slot.append(si)
                break
    assert len(tile_slot) == T
    return slots, tiles, slot_blocks, tile_slot


def _build_program(C, H, M, Tvec, zero_bias):
    NKC = C // TILE          # 8 K-tiles over C
    HCHUNK = H // 8          # 512
    NMH = HCHUNK // TILE     # 4
    NHC = H // HCHUNK        # 8
    HN = H // TILE           # 32 (b1 bias columns)
    S = len(Tvec)
    slots, tiles, slot_blocks, tile_slot = _slot_geometry(Tvec, M)
    T = len(tiles)
    ntok_max = max(n for _, n in slots)

    nc = bass.Bass("TRN2", target_bir_lowering=False, debug=False)
    xc = nc.dram_tensor("xc", [M, C], F32, kind="ExternalInput").ap()
    w1r = nc.dram_tensor("w1r", [S, NHC, TILE, NKC * HCHUNK], BF16,
                         kind="ExternalInput").ap()
    w2r = nc.dram_tensor("w2r", [S, NHC, 2, TILE, NMH * 512], BF16,
                         kind="ExternalInput").ap()
    wcb = nc.dram_tensor("wcb", [S, TILE, C], BF16, kind="ExternalInput").ap()
    bcs = nc.dram_tensor("bcs", [S, TILE, 1], F32, kind="ExternalInput").ap()
    if not zero_bias:
        b1b = nc.dram_tensor("b1b", [S, TILE, HN], F32, kind="ExternalInput").ap()
    yc = nc.dram_tensor("yc", [M, C], F32, kind="ExternalOutput").ap()

    with TileContext(nc) as tc, ExitStack() as ctx:
        cpool = ctx.enter_context(tc.tile_pool(name="const", bufs=1))
        ones = cpool.tile([TILE, TILE], BF16, tag="ones")
        nc.gpsimd.memset(ones[:], 1.0)
        epsc = cpool.tile([TILE, 1], F32, tag="eps")
        nc.gpsimd.memset(epsc[:], LN_EPS)

        xp = ctx.enter_context(tc.tile_pool(name="x", bufs=T))
        hp = ctx.enter_context(tc.tile_pool(name="h", bufs=6))
        sqp = ctx.enter_context(tc.tile_pool(name="sq", bufs=2))
        prp = ctx.enter_context(tc.tile_pool(name="pr", bufs=2))
        stp = ctx.enter_context(tc.tile_pool(name="st", bufs=6))
        scp = ctx.enter_context(tc.tile_pool(name="scps", bufs=T))
        wcbp = ctx.enter_context(tc.tile_pool(name="wcb", bufs=S))
        hTp = ctx.enter_context(tc.tile_pool(name="hT", bufs=1))
        w1p = ctx.enter_context(tc.tile_pool(name="w1", bufs=4))
        w2p = ctx.enter_context(tc.tile_pool(name="w2", bufs=4))
        krp = ctx.enter_context(tc.tile_pool(name="kr", bufs=2))
        ktp = ctx.enter_context(tc.tile_pool(name="kt", bufs=34))
        op = ctx.enter_context(tc.tile_pool(name="o", bufs=4))
        pkp = ctx.enter_context(tc.tile_pool(name="pk", bufs=3, space="PSUM"))
        ysp = ctx.enter_context(tc.tile_pool(name="ys", bufs=5, space="PSUM"))

        # --- upfront SWDGE DMAs: all x tiles, all slot consts -------------
        x_t = []
        for t, (r0, rows) in enumerate(tiles):
            xt = xp.tile([TILE, C], F32, tag="x", name=f"x{t}")
            nc.gpsimd.dma_start(xt[0:rows, :], xc[r0 : r0 + rows, :])
            x_t.append(xt)
        wcb_sb = []
        bct_sb = []
        b1_sb = []
        for s in range(S):
            w = wcbp.tile([TILE, C], BF16, tag="wcb", name=f"wcb{s}")
            nc.gpsimd.dma_start(w[:], wcb[s])
            wcb_sb.append(w)
            b = wcbp.tile([TILE, 1], F32, tag="bc", name=f"bc{s}")
            nc.gpsimd.dma_start(b[:], bcs[s])
            bct_sb.append(b)
            if not zero_bias:
                bb = wcbp.tile([TILE, HN], F32, tag="b1", name=f"b1{s}")
                nc.gpsimd.dma_start(bb[:], b1b[s])
                b1_sb.append(bb)

        # --- PE warm-up (HAM): junk matmuls into the pk pool --------------
        for i in range(WARMUP_MM):
            wm = pkp.tile([TILE, 512], F32, tag="pk", name=f"warm{i}")
            nc.tensor.matmul(wm[:, 0:TILE], ones[:], ones[:], start=True,
                             stop=True)

        # --- per-slot hT buffers ------------------------------------------
        hT = []
        for s, (soff, ntok) in enumerate(slots):
            t_ = hTp.tile([TILE, NKC, ntok], BF16, tag=f"hT{s}", name=f"hT{s}")
            hT.append(t_)

        sc_t = [None] * T

        def emit_stage_a(ts):
            """LayerNorm + confidence + hT transpose for tiles `ts`
            (ops batched by type across the tiles)."""
            n = len(ts)
            st = lambda tag: [
                stp.tile([TILE, 1], F32, tag=tag, name=f"{tag}{t}") for t in ts
            ]
            rows_ = [tiles[t][1] for t in ts]
            nsum, negmu, ssq, std, rs = st("nsum"), st("negmu"), st("ssq"), st("std"), st("rs")
            cdot, conf, cpe, rc = st("cdot"), st("conf"), st("cpe"), st("rc")
            hts = []
            for i, t in enumerate(ts):
                r = rows_[i]
                nc.vector.reduce_sum(
                    nsum[i][0:r], x_t[t][0:r, :], axis=mybir.AxisListType.X,
                    negate=True,
                )
            for i, t in enumerate(ts):
                nc.vector.tensor_scalar_mul(negmu[i][0:rows_[i]],
                                            nsum[i][0:rows_[i]], 1.0 / C)
            if zero_bias:
                # h = x - mu (pre-normalization); 1/std commutes through
                # relu^2 and folds into sc = conf/(conf+eps) * rs^2
                for i, t in enumerate(ts):
                    r = rows_[i]
                    ht_ = hp.tile([TILE, C], BF16, tag="h", name=f"h{t}")
                    hts.append(ht_)
                    nc.scalar.activation(
                        ht_[0:r, :], x_t[t][0:r, :], AF.Identity,
                        bias=negmu[i][0:r], scale=1.0,
                    )
                for i, t in enumerate(ts):
                    r = rows_[i]
                    sq = sqp.tile([TILE, C], F32, tag="sq")
                    nc.scalar.activation(
                        sq[0:r, :], x_t[t][0:r, :], AF.Square,
                        bias=negmu[i][0:r], scale=1.0, accum_out=ssq[i][0:r],
                    )
                for i, t in enumerate(ts):
                    r = rows_[i]
                    nc.scalar.activation(
                        std[i][0:r], ssq[i][0:r], AF.Sqrt, bias=epsc[0:r],
                        scale=1.0 / C,
                    )
                # hT via XBAR DMA-transpose (scalar queue: ordered after h)
                for i, t in enumerate(ts):
                    r = rows_[i]
                    s = tile_slot[t]
                    c0 = tiles[t][0] - slots[s][0]
                    nc.scalar.dma_start(
                        hT[s][:, :, c0 : c0 + r], hts[i][0:r, :],
                        transpose=True,
                    )
                for i, t in enumerate(ts):
                    nc.vector.reciprocal(rs[i][0:rows_[i]], std[i][0:rows_[i]])
                for i, t in enumerate(ts):
                    r = rows_[i]
                    prod = prp.tile([TILE, C], BF16, tag="prod")
                    nc.vector.scalar_tensor_tensor(
                        prod[0:r, :], hts[i][0:r, :], 1.0,
                        wcb_sb[tile_slot[t]][0:r, :], op0=OP.mult,
                        op1=OP.mult, accum_out=cdot[i][0:r],
                    )
                for i, t in enumerate(ts):
                    r = rows_[i]
                    nc.scalar.activation(
                        conf[i][0:r], cdot[i][0:r], AF.Sigmoid,
                        bias=bct_sb[tile_slot[t]][0:r], scale=rs[i][0:r],
                    )
                for i, t in enumerate(ts):
                    r = rows_[i]
                    nc.vector.tensor_scalar_add(cpe[i][0:r], conf[i][0:r], 1e-6)
                for i, t in enumerate(ts):
                    nc.vector.reciprocal(rc[i][0:rows_[i]], cpe[i][0:rows_[i]])
                rs2, sc0 = st("rs2"), st("sc0")
                for i, t in enumerate(ts):
                    r = rows_[i]
                    nc.vector.tensor_mul(rs2[i][0:r], rs[i][0:r], rs[i][0:r])
                for i, t in enumerate(ts):
                    r = rows_[i]
                    nc.vector.tensor_mul(sc0[i][0:r], conf[i][0:r], rc[i][0:r])
                for i, t in enumerate(ts):
                    r = rows_[i]
                    sc = scp.tile([TILE, 1], F32, tag="sc", name=f"sc{t}")
                    nc.vector.tensor_mul(sc[0:r], sc0[i][0:r], rs2[i][0:r])
                    sc_t[t] = sc
                return
            # general path (beta != 0): h fully normalized
            nmrs = st("nmrs")
            for i, t in enumerate(ts):
                r = rows_[i]
                sq = sqp.tile([TILE, C], F32, tag="sq")
                nc.scalar.activation(
                    sq[0:r, :], x_t[t][0:r, :], AF.Square, bias=negmu[i][0:r],
                    scale=1.0, accum_out=ssq[i][0:r],
                )
            for i, t in enumerate(ts):
                r = rows_[i]
                nc.scalar.activation(
                    std[i][0:r], ssq[i][0:r], AF.Sqrt, bias=epsc[0:r],
                    scale=1.0 / C,
                )
            for i, t in enumerate(ts):
                nc.vector.reciprocal(rs[i][0:rows_[i]], std[i][0:rows_[i]])
            for i, t in enumerate(ts):
                r = rows_[i]
                nc.vector.tensor_mul(nmrs[i][0:r], negmu[i][0:r], rs[i][0:r])
            for i, t in enumerate(ts):
                r = rows_[i]
                ht_ = hp.tile([TILE, C], BF16, tag="h", name=f"h{t}")
                hts.append(ht_)
                nc.scalar.activation(
                    ht_[0:r, :], x_t[t][0:r, :], AF.Identity,
                    bias=nmrs[i][0:r], scale=rs[i][0:r],
                )
            for i, t in enumerate(ts):
                r = rows_[i]
                s = tile_slot[t]
                c0 = tiles[t][0] - slots[s][0]
                nc.scalar.dma_start(
                    hT[s][:, :, c0 : c0 + r], hts[i][0:r, :], transpose=True,
                )
            for i, t in enumerate(ts):
                r = rows_[i]
                prod = prp.tile([TILE, C], BF16, tag="prod")
                nc.vector.scalar_tensor_tensor(
                    prod[0:r, :], hts[i][0:r, :], 1.0,
                    wcb_sb[tile_slot[t]][0:r, :], op0=OP.mult, op1=OP.mult,
                    accum_out=cdot[i][0:r],
                )
            for i, t in enumerate(ts):
                r = rows_[i]
                nc.scalar.activation(
                    conf[i][0:r], cdot[i][0:r], AF.Sigmoid,
                    bias=bct_sb[tile_slot[t]][0:r], scale=1.0,
                )
            for i, t in enumerate(ts):
                r = rows_[i]
                nc.vector.tensor_scalar_add(cpe[i][0:r], conf[i][0:r], 1e-6)
            for i, t in enumerate(ts):
                nc.vector.reciprocal(rc[i][0:rows_[i]], cpe[i][0:rows_[i]])
            for i, t in enumerate(ts):
                r = rows_[i]
                sc = scp.tile([TILE, 1], F32, tag="sc", name=f"sc{t}")
                nc.vector.tensor_mul(sc[0:r], conf[i][0:r], rc[i][0:r])
                sc_t[t] = sc

        # stage-A scheduling: slot0's tiles upfront; remaining tiles fed
        # one-at-a-time at matmul loop hook points so the DVE queue never
        # gets a burst that stalls the relu^2 stream.
        pending_tiles = [t for t in range(T)]
        first_batch = [t for t in pending_tiles if tile_slot[t] == 0]
        emit_stage_a(first_batch)
        pending_tiles = [t for t in pending_tiles if tile_slot[t] != 0]

        def stage_a_hook():
            if pending_tiles:
                emit_stage_a([pending_tiles.pop(0)])

        def ensure_tiles_for_slot(s):
            need = [t for t in pending_tiles if tile_slot[t] == s]
            if need:
                emit_stage_a(need)
                for t in need:
                    pending_tiles.remove(t)

        kt_tiles = {}

        for s in range(S):
            soff, ntok = slots[s]
            ensure_tiles_for_slot(s)
            # mm1 pk chunks: <=512 moving cols each (one PSUM bank), all
            # 64-multiples and as even as possible so none is LDW-bound
            q = ntok // QT
            nck = (q + 7) // 8
            base, rem = divmod(q, nck)
            ck_parts = [(base + (1 if i < rem else 0)) * QT for i in range(nck)]
            ck_offs = [sum(ck_parts[:i]) for i in range(nck)]
            # ---- mm1 phase: kt[hc,mh] = relu(W1c^T @ hT)^2 ---------------
            for hc in range(NHC):
                w1t = w1p.tile([TILE, NKC * HCHUNK], BF16, tag="w1",
                               name=f"w1_{s}_{hc}")
                nc.sync.dma_start(w1t[:], w1r[s, hc])
                if hc % 2 == 1:
                    stage_a_hook()
                for mh in range(NMH):
                    kt = ktp.tile([TILE, ntok_max], BF16, tag="kt",
                                  name=f"kt{s}_{hc}_{mh}")
                    for co, cw in zip(ck_offs, ck_parts):
                        pk = pkp.tile([TILE, 512], F32, tag="pk")
                        for kc in range(NKC):
                            nc.tensor.matmul(
                                pk[:, 0:cw],
                                w1t[:, kc * HCHUNK + mh * TILE : kc * HCHUNK + (mh + 1) * TILE],
                                hT[s][:, kc, co : co + cw],
                                start=(kc == 0),
                                stop=(kc == NKC - 1),
                            )
                        kr = krp.tile([TILE, 512], BF16, tag="kr")
                        if zero_bias:
                            nc.vector.tensor_scalar_max(kr[:, 0:cw], pk[:, 0:cw], 0.0)
                        else:
                            col = hc * NMH + mh
                            nc.scalar.activation(
                                kr[:, 0:cw], pk[:, 0:cw], AF.Relu,
                                bias=b1_sb[s][:, col : col + 1], scale=1.0,
                            )
                        nc.vector.tensor_mul(
                            kt[:, co : co + cw], kr[:, 0:cw], kr[:, 0:cw]
                        )
                    kt_tiles[(hc, mh)] = kt
            # ---- mm2 phase: two C-halves, ys[block] one psum bank each ---
            for half in range(2):
                ys_b = {}
                for hc in range(NHC):
                    w2t = w2p.tile([TILE, NMH * 512], BF16, tag="w2",
                                   name=f"w2_{s}_{hc}_{half}")
                    nc.sync.dma_start(w2t[:], w2r[s, hc, half])
                    if hc % 3 == 2:
                        stage_a_hook()
                    for mh in range(NMH):
                        kt = kt_tiles[(hc, mh)]
                        for (c0, bw, gt) in slot_blocks[s]:
                            if (hc, mh) == (0, 0):
                                ys_b[c0] = ysp.tile([TILE, 512], F32, tag="ys",
                                                    name=f"ys{s}_{half}_{c0}")
                            nc.tensor.matmul(
                                ys_b[c0][0:bw, :],
                                kt[:, c0 : c0 + bw],
                                w2t[:, mh * 512 : (mh + 1) * 512],
                                start=(hc == 0 and mh == 0),
                                stop=(hc == NHC - 1 and mh == NMH - 1),
                            )
                # combine + output DMA per block half
                for (c0, bw, gt) in slot_blocks[s]:
                    ot = op.tile([TILE, 512], F32, tag="o")
                    nc.vector.scalar_tensor_tensor(
                        ot[0:bw, :],
                        ys_b[c0][0:bw, :],
                        sc_t[gt][0:bw],
                        x_t[gt][0:bw, half * 512 : (half + 1) * 512],
                        op0=OP.mult,
                        op1=OP.add,
                    )
                    r0 = soff + c0
                    nc.gpsimd.dma_start(
                        yc[r0 : r0 + bw, half * 512 : (half + 1) * 512],
                        ot[0:bw, :],
                    )

    _split_excess_waits(nc, 1)
    return nc


# ---------------------------------------------------------------------------
# Host-side dispatch
# ---------------------------------------------------------------------------


MAXQ = 10  # max slot size in quanta (<=5 mm2 blocks -> 5 ys PSUM banks)


def _even_partitions(total, odd_last, min_part, max_len):
    """Yield tuples of slot sizes (quanta): all even except (iff odd_last)
    the final element which is odd; each >= min_part; ascending evens first."""
    def parts_even(rem, max_first, length):
        if rem == 0:
            yield ()
            return
        if length == 0:
            return
        f = min(rem, max_first, MAXQ)
        if f % 2 == 1:
            f -= 1
        while f >= min_part:
            for rest in parts_even(rem - f, f, length - 1):
                yield (f,) + rest
            f -= 2

    if odd_last:
        for last in range(min_part | 1, min(total, MAXQ) + 1, 2):
            rem = total - last
            if rem == 0:
                yield (last,)
                continue
            for ev in parts_even(rem, rem, max_len - 1):
                yield tuple(sorted(ev)) + (last,)
    else:
        for ev in parts_even(total, total, max_len):
            yield tuple(sorted(ev))


def _try_pack_q(qe, Tvec):
    """Greedy assign experts (quanta counts qe) to slot instances
    (8 per slot size). Returns assign[i] = [(slot_j, core_c), ...] or None."""
    avail = [list(range(NCORES)) for _ in Tvec]
    order_i = sorted(range(len(qe)), key=lambda i: -qe[i])
    assign = [None] * len(qe)
    sizes = sorted(range(len(Tvec)), key=lambda j: -Tvec[j])
    for i in order_i:
        rem = qe[i]
        inst = []
        while rem > 0:
            pick = None
            for j in sizes:
                if avail[j] and Tvec[j] <= rem:
                    pick = j
                    break
            if pick is None:
                for j in reversed(sizes):
                    if avail[j]:
                        pick = j
                        break
            if pick is None:
                return None
            c = avail[pick].pop(0)
            inst.append((pick, c))
            rem -= Tvec[pick]
        assign[i] = inst
    return assign


def _pack_slots_q(qe):
    """Choose per-core slot sizes (quanta, 128-aligned boundaries: evens
    then possibly one odd last) and expert->instance assignment."""
    total = sum(qe)
    Q0 = int(math.ceil(total / NCORES))
    for Q in range(Q0, Q0 + 6):
        odd_last = (Q % 2 == 1)
        cands = sorted(
            set(_even_partitions(Q, odd_last, 3 if odd_last else 2, 5)),
            key=lambda tv: (len(tv), -min(tv)),
        )
        # prefer all parts >= 4 quanta (mm1 stays moving-bound)
        for min4 in (True, False):
            for Tvec in cands:
                if min4 and min(Tvec) < 4:
                    continue
                a = _try_pack_q(list(qe), list(Tvec))
                if a is not None:
                    return list(Tvec), a
    raise RuntimeError(f"no packing found for quanta {qe}")


def _prepare(x, winners, gamma, beta, w1, w2, wc, bc):
    x = np.ascontiguousarray(np.asarray(x, dtype=np.float32))
    winners = np.asarray(winners).reshape(-1).astype(np.int64)
    gamma = np.asarray(gamma, dtype=np.float32)
    beta = np.asarray(beta, dtype=np.float32)
    w1 = np.asarray(w1, dtype=np.float32)
    w2 = np.asarray(w2, dtype=np.float32)
    wc = np.asarray(wc, dtype=np.float32)
    bc = np.asarray(bc, dtype=np.float32)

    B, T_, C = x.shape
    E, _, H = w1.shape
    N = B * T_
    xf = x.reshape(N, C)
    NKC = C // TILE
    HCHUNK = H // 8
    NMH = HCHUNK // TILE
    NHC = H // HCHUNK

    order = np.argsort(winners, kind="stable")
    counts = np.bincount(winners, minlength=E)
    present = [e for e in range(E) if counts[e] > 0]
    qe = [int(math.ceil(counts[e] / QT)) for e in present]

    Tvec, assign = _pack_slots_q(qe)
    S = len(Tvec)
    M = sum(Tvec) * QT

    slot_expert = [[present[0]] * S for _ in range(NCORES)]
    slot_idx = [
        [np.full(Tvec[j] * QT, -1, dtype=np.int64) for j in range(S)]
        for c in range(NCORES)
    ]
    pos = 0
    for i, e in enumerate(present):
        n_e = int(counts[e])
        toks = order[pos : pos + n_e]
        pos += n_e
        filled = 0
        for (j, c) in assign[i]:
            slot_expert[c][j] = e
            cap = Tvec[j] * QT
            take = min(cap, n_e - filled)
            if take > 0:
                slot_idx[c][j][:take] = toks[filled : filled + take]
                filled += take
        assert filled == n_e
    per_core_idx = [np.concatenate(slot_idx[c]) for c in range(NCORES)]

    zero_bias = bool(np.all(beta == 0.0))
    w1r_e, w2r_e, wcb_e, bc_e, b1b_e = {}, {}, {}, {}, {}
    for e in present:
        w1f = (w1[e] * gamma[:, None]).astype(NP_BF16)
        w2f = w2[e].astype(NP_BF16)
        # w1 chunk layout: [NHC, TILE(c within kc), NKC*HCHUNK]
        w1r_e[e] = np.ascontiguousarray(
            w1f.reshape(NKC, TILE, NHC, HCHUNK)
            .transpose(2, 1, 0, 3)
            .reshape(NHC, TILE, NKC * HCHUNK)
        )
        # w2 half-chunk layout: [NHC, 2(half), TILE(h within mh), NMH*512]
        w2r_e[e] = np.ascontiguousarray(
            w2f.reshape(NHC, NMH, TILE, 2, 512)
            .transpose(0, 3, 2, 1, 4)
            .reshape(NHC, 2, TILE, NMH * 512)
        )
        wcf = (wc[e] * gamma).astype(NP_BF16)
        wcb_e[e] = np.ascontiguousarray(np.broadcast_to(wcf[None, :], (TILE, C)))
        bc_e[e] = np.full((TILE, 1), float(bc[e] + float(beta @ wc[e])),
                          dtype=np.float32)
        if not zero_bias:
            b1 = beta @ w1[e]
            b1b_e[e] = np.ascontiguousarray(
                b1.reshape(H // TILE, TILE).T
            ).astype(np.float32)

    in_maps = []
    for c in range(NCORES):
        idx = per_core_idx[c]
        xcrows = np.zeros((M, C), dtype=np.float32)
        valid = idx >= 0
        xcrows[valid] = xf[idx[valid]]
        sl = slot_expert[c]
        m = {
            "xc": xcrows,
            "w1r": np.stack([w1r_e[e] for e in sl]),
            "w2r": np.stack([w2r_e[e] for e in sl]),
            "wcb": np.stack([wcb_e[e] for e in sl]),
            "bcs": np.stack([bc_e[e] for e in sl]),
        }
        if not zero_bias:
            m["b1b"] = np.stack([b1b_e[e] for e in sl])
        in_maps.append(m)

    meta = dict(
        B=B, T=T_, C=C, H=H, N=N, M=M, Tvec=Tvec, zero_bias=zero_bias,
        per_core_idx=per_core_idx,
    )
    return in_maps, meta


def _assemble(results, meta):
    N, C = meta["N"], meta["C"]
    out = np.empty((N, C), dtype=np.float32)
    seen = np.zeros(N, dtype=bool)
    for c in range(NCORES):
        idx = meta["per_core_idx"][c]
        valid = idx >= 0
        out[idx[valid]] = results[c]["yc"][valid]
        seen[idx[valid]] = True
    assert seen.all()
    return out.reshape(meta["B"], meta["T"], C)


def kernel_with_results(x, winners, gamma, beta, w1, w2, wc, bc, **run_kwargs):
    in_maps, meta = _prepare(x, winners, gamma, beta, w1, w2, wc, bc)
    nc = _build_program(
        meta["C"], meta["H"], meta["M"], meta["Tvec"], meta["zero_bias"]
    )
    res = run_bass_kernel_spmd(nc, in_maps, core_ids=list(range(NCORES)), **run_kwargs)
    return _assemble(res.results, meta), res


def kernel(x, winners, gamma, beta, w1, w2, wc, bc):
    out, _ = kernel_with_results(x, winners, gamma, beta, w1, w2, wc, bc)
    return out


# revision 7
# speedup vs baseline: 1.1508x; 1.1508x over previous
"""CaMoE block (LayerNorm -> per-expert squared-ReLU FFN with top-1 routing,
confidence-scaled combine, residual) on 8 Trainium2 NeuronCores.

v2 strategy (64-token quanta, slot-granular matmul phases):
  * Host: stable-sort tokens by winning expert; pack into per-core SLOTS
    sized in 64-token quanta, identical slot structure across cores (SPMD).
    All slot sizes are even (multiples of 128 tokens) except possibly the
    last, so every slot boundary is 128-aligned; per-core token count is
    64*ceil(sum_e ceil(cnt_e/64) / 8) -- less padding than 128-tile packing.
  * Device (per core):
      stage A (per 128-token tile): LayerNorm stats + confidence
        sigmoid(rs*(h.wc)+bc) and straight-through scale
        sc = conf/(conf+1e-6) * rs^2 (1/std folded out of h, it commutes
        through relu^2 into the final scale); h = x - mu in bf16;
        hT built by XBAR DMA-transpose (no PE/PSUM involvement).
      per slot: mm1 phase (W1 chunks streamed, pk = W1c^T @ hT over all
        slot tokens at once, 256..448 moving cols -> LDWEIGHTS fully
        hidden), relu^2 on DVE -> kt (whole slot resident in SBUF);
        mm2 phase in two C-halves (ys[block, half] psum is one bank each):
        ys += kt_block^T @ W2chunk, combine ot = ys*sc + x, DMA out.
  * PE warm-up matmuls at program start keep the HAM clock at 2.4 GHz by
    the time the first real matmul issues.
  * DMA submissions split across queues: sync=weight streams,
    scalar(ACT)=hT transposes (ordered after the h they read),
    gpsimd(SWDGE)=x/wcb/bc/out.
  * Host: scatter rows back to original token positions.

gamma/beta are folded into W1/wc on the host (beta==0 fast path assumed,
with a general fallback mirroring the math via an H-bias).
Matmuls in bf16 with fp32 PSUM accumulation.
"""

import math
import os
from contextlib import ExitStack

import numpy as np

import concourse.bass as bass
import concourse.mybir as mybir
import concourse.tile as tile
from concourse.bass_utils import run_bass_kernel_spmd
from concourse.tile import TileContext, ScopedClock

AF = mybir.ActivationFunctionType
OP = mybir.AluOpType
BF16 = mybir.dt.bfloat16
F32 = mybir.dt.float32
NP_BF16 = mybir.dt.np(BF16)

NCORES = 8
TILE = 128
QT = 64                  # token quantum
LN_EPS = 1e-5
WARMUP_MM = 64

# ---------------------------------------------------------------------------
# Walrus workarounds (single-wait encoding), same as v1.
# ---------------------------------------------------------------------------


def _patched_drain_and_barrier(self, tick_clock, wait_clock):
    probe = self.nc.sync.nop(nofuse=True)
    wait_clock.add_sem_waits(probe.ins, ScopedClock({None: tick_clock.global_clock}))
    si = probe.ins.sync_info
    waits = list(si.on_wait) if si is not None and si.on_wait else []
    if len(waits) > 1:
        probe.ins.sync_info = mybir.SyncInfo(on_wait=[waits[0]], on_update=[])
        for w in waits[1:]:
            n = self.nc.sync.nop(nofuse=True)
            n.ins.sync_info = mybir.SyncInfo(on_wait=[w], on_update=[])
    self.nc.sync.drain()
    self.nc.all_engine_barrier()
    assert self.sems is not None
    popped = self.nc._tile_sem_poison_stack.pop()
    assert popped is self._sem_poison
    self.nc.clear_and_free_semaphores(list(self.sems.allocated().values()))
    self.nc.all_engine_barrier()


TileContext._drain_and_barrier = _patched_drain_and_barrier


def _split_excess_waits(nc, max_waits: int = 1):
    for fn in nc.m.functions:
        for bb in fn.blocks:
            insts = list(bb.instructions)
            out = []
            changed = False
            for inst in insts:
                si = inst.sync_info
                waits = list(si.on_wait) if si is not None and si.on_wait else []
                if len(waits) > max_waits:
                    extra = waits[:-max_waits]
                    keep = waits[-max_waits:]
                    for j, w in enumerate(extra):
                        nop = mybir.InstNoOp(
                            name=f"{inst.name}-wsplit{j}", ins=[], outs=[]
                        )
                        nop.engine = inst.engine
                        nop.sync_info = mybir.SyncInfo(on_wait=[w], on_update=[])
                        out.append(nop)
                    inst.sync_info = mybir.SyncInfo(
                        on_wait=keep,
                        on_update=list(si.on_update) if si.on_update else [],
                    )
                    changed = True
                out.append(inst)
            if changed:
                bb.instructions = out


# ---------------------------------------------------------------------------
# Device program
# ---------------------------------------------------------------------------


def _slot_geometry(Tvec, M):
    """Per-slot (tok_off, ntok), LN tiles (global 128-blocks), per-slot
    mm2 blocks [(col_off_in_slot, width, global_tile_idx), ...]."""
    slots = []
    off = 0
    for q in Tvec:
        slots.append((off, q * QT))
        off += q * QT
    assert off == M
    T = (M + TILE - 1) // TILE
    tiles = [(t * TILE, min(TILE, M - t * TILE)) for t in range(T)]
    slot_blocks = []
    for (soff, ntok) in slots:
        blocks = []
        c = 0
        while c < ntok:
            w = min(TILE, ntok - c)
            gt = (soff + c) // TILE
            assert (soff + c) % TILE == 0
            blocks.append((c, w, gt))
            c += w
        slot_blocks.append(blocks)
    # tile -> slot
    tile_slot = []
    for (r0, _rows) in tiles:
        for si, (soff, ntok) in enumerate(slots):
            if soff <= r0 < soff + ntok:
                tile_slot.append(si)
                break
    assert len(tile_slot) == T
    return slots, tiles, slot_blocks, tile_slot


def _build_program(C, H, M, Tvec, zero_bias):
    NKC = C // TILE          # 8 K-tiles over C
    HCHUNK = H // 8          # 512
    NMH = HCHUNK // TILE     # 4
    NHC = H // HCHUNK        # 8
    HN = H // TILE           # 32 (b1 bias columns)
    S = len(Tvec)
    slots, tiles, slot_blocks, tile_slot = _slot_geometry(Tvec, M)
    T = len(tiles)
    ntok_max = max(n for _, n in slots)

    nc = bass.Bass("TRN2", target_bir_lowering=False, debug=False)
    xc = nc.dram_tensor("xc", [M, C], F32, kind="ExternalInput").ap()
    w1r = nc.dram_tensor("w1r", [S, NHC, TILE, NKC * HCHUNK], BF16,
                         kind="ExternalInput").ap()
    w2r = nc.dram_tensor("w2r", [S, NHC, 2, TILE, NMH * 512], BF16,
                         kind="ExternalInput").ap()
    wcb = nc.dram_tensor("wcb", [S, TILE, C], BF16, kind="ExternalInput").ap()
    bcs = nc.dram_tensor("bcs", [S, TILE, 1], F32, kind="ExternalInput").ap()
    if not zero_bias:
        b1b = nc.dram_tensor("b1b", [S, TILE, HN], F32, kind="ExternalInput").ap()
    yc = nc.dram_tensor("yc", [M, C], F32, kind="ExternalOutput").ap()

    with TileContext(nc) as tc, ExitStack() as ctx:
        cpool = ctx.enter_context(tc.tile_pool(name="const", bufs=1))
        ones = cpool.tile([TILE, TILE], BF16, tag="ones")
        nc.gpsimd.memset(ones[:], 1.0)
        epsc = cpool.tile([TILE, 1], F32, tag="eps")
        nc.gpsimd.memset(epsc[:], LN_EPS)

        xp = ctx.enter_context(tc.tile_pool(name="x", bufs=T))
        hp = ctx.enter_context(tc.tile_pool(name="h", bufs=6))
        sqp = ctx.enter_context(tc.tile_pool(name="sq", bufs=2))
        prp = ctx.enter_context(tc.tile_pool(name="pr", bufs=2))
        stp = ctx.enter_context(tc.tile_pool(name="st", bufs=6))
        scp = ctx.enter_context(tc.tile_pool(name="scps", bufs=T))
        wcbp = ctx.enter_context(tc.tile_pool(name="wcb", bufs=S))
        hTp = ctx.enter_context(tc.tile_pool(name="hT", bufs=1))
        w1p = ctx.enter_context(tc.tile_pool(name="w1", bufs=4))
        w2p = ctx.enter_context(tc.tile_pool(name="w2", bufs=4))
        krp = ctx.enter_context(tc.tile_pool(name="kr", bufs=2))
        ktp = ctx.enter_context(tc.tile_pool(name="kt", bufs=34))
        op = ctx.enter_context(tc.tile_pool(name="o", bufs=4))
        pkp = ctx.enter_context(tc.tile_pool(name="pk", bufs=3, space="PSUM"))
        ysp = ctx.enter_context(tc.tile_pool(name="ys", bufs=5, space="PSUM"))

        # --- upfront SWDGE DMAs: all x tiles, all slot consts -------------
        x_t = []
        for t, (r0, rows) in enumerate(tiles):
            xt = xp.tile([TILE, C], F32, tag="x", name=f"x{t}")
            nc.scalar.dma_start(xt[0:rows, :], xc[r0 : r0 + rows, :])
            x_t.append(xt)
        wcb_sb = []
        bct_sb = []
        b1_sb = []
        for s in range(S):
            w = wcbp.tile([TILE, C], BF16, tag="wcb", name=f"wcb{s}")
            nc.scalar.dma_start(w[:], wcb[s])
            wcb_sb.append(w)
            b = wcbp.tile([TILE, 1], F32, tag="bc", name=f"bc{s}")
            nc.scalar.dma_start(b[:], bcs[s])
            bct_sb.append(b)
            if not zero_bias:
                bb = wcbp.tile([TILE, HN], F32, tag="b1", name=f"b1{s}")
                nc.scalar.dma_start(bb[:], b1b[s])
                b1_sb.append(bb)

        # --- PE warm-up (HAM): junk matmuls into the pk pool --------------
        for i in range(WARMUP_MM):
            wm = pkp.tile([TILE, 512], F32, tag="pk", name=f"warm{i}")
            nc.tensor.matmul(wm[:, 0:TILE], ones[:], ones[:], start=True,
                             stop=True)

        # --- per-slot hT buffers ------------------------------------------
        hT = []
        for s, (soff, ntok) in enumerate(slots):
            t_ = hTp.tile([TILE, NKC, ntok], BF16, tag=f"hT{s}", name=f"hT{s}")
            hT.append(t_)

        sc_t = [None] * T

        def emit_stage_a(ts):
            """LayerNorm + confidence + hT transpose for tiles `ts`
            (ops batched by type across the tiles)."""
            n = len(ts)
            st = lambda tag: [
                stp.tile([TILE, 1], F32, tag=tag, name=f"{tag}{t}") for t in ts
            ]
            rows_ = [tiles[t][1] for t in ts]
            nsum, negmu, ssq, std, rs = st("nsum"), st("negmu"), st("ssq"), st("std"), st("rs")
            cdot, conf, cpe, rc = st("cdot"), st("conf"), st("cpe"), st("rc")
            hts = []
            for i, t in enumerate(ts):
                r = rows_[i]
                nc.vector.reduce_sum(
                    nsum[i][0:r], x_t[t][0:r, :], axis=mybir.AxisListType.X,
                    negate=True,
                )
            for i, t in enumerate(ts):
                nc.vector.tensor_scalar_mul(negmu[i][0:rows_[i]],
                                            nsum[i][0:rows_[i]], 1.0 / C)
            if zero_bias:
                # h = x - mu (pre-normalization); 1/std commutes through
                # relu^2 and folds into sc = conf/(conf+eps) * rs^2
                for i, t in enumerate(ts):
                    r = rows_[i]
                    ht_ = hp.tile([TILE, C], BF16, tag="h", name=f"h{t}")
                    hts.append(ht_)
                    nc.scalar.activation(
                        ht_[0:r, :], x_t[t][0:r, :], AF.Identity,
                        bias=negmu[i][0:r], scale=1.0,
                    )
                for i, t in enumerate(ts):
                    r = rows_[i]
                    sq = sqp.tile([TILE, C], F32, tag="sq")
                    nc.scalar.activation(
                        sq[0:r, :], x_t[t][0:r, :], AF.Square,
                        bias=negmu[i][0:r], scale=1.0, accum_out=ssq[i][0:r],
                    )
                for i, t in enumerate(ts):
                    r = rows_[i]
                    nc.scalar.activation(
                        std[i][0:r], ssq[i][0:r], AF.Sqrt, bias=epsc[0:r],
                        scale=1.0 / C,
                    )
                # hT via XBAR DMA-transpose (scalar queue: ordered after h)
                for i, t in enumerate(ts):
                    r = rows_[i]
                    s = tile_slot[t]
                    c0 = tiles[t][0] - slots[s][0]
                    nc.scalar.dma_start(
                        hT[s][:, :, c0 : c0 + r], hts[i][0:r, :],
                        transpose=True,
                    )
                for i, t in enumerate(ts):
                    nc.vector.reciprocal(rs[i][0:rows_[i]], std[i][0:rows_[i]])
                for i, t in enumerate(ts):
                    r = rows_[i]
                    prod = prp.tile([TILE, C], BF16, tag="prod")
                    nc.vector.scalar_tensor_tensor(
                        prod[0:r, :], hts[i][0:r, :], 1.0,
                        wcb_sb[tile_slot[t]][0:r, :], op0=OP.mult,
                        op1=OP.mult, accum_out=cdot[i][0:r],
                    )
                for i, t in enumerate(ts):
                    r = rows_[i]
                    nc.scalar.activation(
                        conf[i][0:r], cdot[i][0:r], AF.Sigmoid,
                        bias=bct_sb[tile_slot[t]][0:r], scale=rs[i][0:r],
                    )
                for i, t in enumerate(ts):
                    r = rows_[i]
                    nc.vector.tensor_scalar_add(cpe[i][0:r], conf[i][0:r], 1e-6)
                for i, t in enumerate(ts):
                    nc.vector.reciprocal(rc[i][0:rows_[i]], cpe[i][0:rows_[i]])
                rs2, sc0 = st("rs2"), st("sc0")
                for i, t in enumerate(ts):
                    r = rows_[i]
                    nc.vector.tensor_mul(rs2[i][0:r], rs[i][0:r], rs[i][0:r])
                for i, t in enumerate(ts):
                    r = rows_[i]
                    nc.vector.tensor_mul(sc0[i][0:r], conf[i][0:r], rc[i][0:r])
                for i, t in enumerate(ts):
                    r = rows_[i]
                    sc = scp.tile([TILE, 1], F32, tag="sc", name=f"sc{t}")
                    nc.vector.tensor_mul(sc[0:r], sc0[i][0:r], rs2[i][0:r])
                    sc_t[t] = sc
                return
            # general path (beta != 0): h fully normalized
            nmrs = st("nmrs")
            for i, t in enumerate(ts):
                r = rows_[i]
                sq = sqp.tile([TILE, C], F32, tag="sq")
                nc.scalar.activation(
                    sq[0:r, :], x_t[t][0:r, :], AF.Square, bias=negmu[i][0:r],
                    scale=1.0, accum_out=ssq[i][0:r],
                )
            for i, t in enumerate(ts):
                r = rows_[i]
                nc.scalar.activation(
                    std[i][0:r], ssq[i][0:r], AF.Sqrt, bias=epsc[0:r],
                    scale=1.0 / C,
                )
            for i, t in enumerate(ts):
                nc.vector.reciprocal(rs[i][0:rows_[i]], std[i][0:rows_[i]])
            for i, t in enumerate(ts):
                r = rows_[i]
                nc.vector.tensor_mul(nmrs[i][0:r], negmu[i][0:r], rs[i][0:r])
            for i, t in enumerate(ts):
                r = rows_[i]
                ht_ = hp.tile([TILE, C], BF16, tag="h", name=f"h{t}")
                hts.append(ht_)
                nc.scalar.activation(
                    ht_[0:r, :], x_t[t][0:r, :], AF.Identity,
                    bias=nmrs[i][0:r], scale=rs[i][0:r],
                )
            for i, t in enumerate(ts):
                r = rows_[i]
                s = tile_slot[t]
                c0 = tiles[t][0] - slots[s][0]
                nc.scalar.dma_start(
                    hT[s][:, :, c0 : c0 + r], hts[i][0:r, :], transpose=True,
                )
            for i, t in enumerate(ts):
                r = rows_[i]
                prod = prp.tile([TILE, C], BF16, tag="prod")
                nc.vector.scalar_tensor_tensor(
                    prod[0:r, :], hts[i][0:r, :], 1.0,
                    wcb_sb[tile_slot[t]][0:r, :], op0=OP.mult, op1=OP.mult,
                    accum_out=cdot[i][0:r],
                )
            for i, t in enumerate(ts):
                r = rows_[i]
                nc.scalar.activation(
                    conf[i][0:r], cdot[i][0:r], AF.Sigmoid,
                    bias=bct_sb[tile_slot[t]][0:r], scale=1.0,
                )
            for i, t in enumerate(ts):
                r = rows_[i]
                nc.vector.tensor_scalar_add(cpe[i][0:r], conf[i][0:r], 1e-6)
            for i, t in enumerate(ts):
                nc.vector.reciprocal(rc[i][0:rows_[i]], cpe[i][0:rows_[i]])
            for i, t in enumerate(ts):
                r = rows_[i]
                sc = scp.tile([TILE, 1], F32, tag="sc", name=f"sc{t}")
                nc.vector.tensor_mul(sc[0:r], conf[i][0:r], rc[i][0:r])
                sc_t[t] = sc

        # stage-A scheduling: slot0's tiles upfront; remaining tiles fed
        # one-at-a-time at matmul loop hook points so the DVE queue never
        # gets a burst that stalls the relu^2 stream.
        pending_tiles = [t for t in range(T)]
        first_batch = [t for t in pending_tiles if tile_slot[t] == 0]
        emit_stage_a(first_batch)
        pending_tiles = [t for t in pending_tiles if tile_slot[t] != 0]

        def stage_a_hook():
            if pending_tiles:
                emit_stage_a([pending_tiles.pop(0)])

        def ensure_tiles_for_slot(s):
            need = [t for t in pending_tiles if tile_slot[t] == s]
            if need:
                emit_stage_a(need)
                for t in need:
                    pending_tiles.remove(t)

        kt_tiles = {}

        for s in range(S):
            soff, ntok = slots[s]
            ensure_tiles_for_slot(s)
            # mm1 pk chunks: <=512 moving cols each (one PSUM bank), all
            # 64-multiples and as even as possible so none is LDW-bound
            q = ntok // QT
            nck = (q + 7) // 8
            base, rem = divmod(q, nck)
            ck_parts = [(base + (1 if i < rem else 0)) * QT for i in range(nck)]
            ck_offs = [sum(ck_parts[:i]) for i in range(nck)]
            # ---- mm1 phase: kt[hc,mh] = relu(W1c^T @ hT)^2 ---------------
            for hc in range(NHC):
                w1t = w1p.tile([TILE, NKC * HCHUNK], BF16, tag="w1",
                               name=f"w1_{s}_{hc}")
                nc.sync.dma_start(w1t[:], w1r[s, hc])
                if hc % 2 == 1:
                    stage_a_hook()
                for mh in range(NMH):
                    kt = ktp.tile([TILE, ntok_max], BF16, tag="kt",
                                  name=f"kt{s}_{hc}_{mh}")
                    for co, cw in zip(ck_offs, ck_parts):
                        pk = pkp.tile([TILE, 512], F32, tag="pk")
                        for kc in range(NKC):
                            nc.tensor.matmul(
                                pk[:, 0:cw],
                                w1t[:, kc * HCHUNK + mh * TILE : kc * HCHUNK + (mh + 1) * TILE],
                                hT[s][:, kc, co : co + cw],
                                start=(kc == 0),
                                stop=(kc == NKC - 1),
                            )
                        kr = krp.tile([TILE, 512], BF16, tag="kr")
                        if zero_bias:
                            nc.vector.tensor_scalar_max(kr[:, 0:cw], pk[:, 0:cw], 0.0)
                        else:
                            col = hc * NMH + mh
                            nc.scalar.activation(
                                kr[:, 0:cw], pk[:, 0:cw], AF.Relu,
                                bias=b1_sb[s][:, col : col + 1], scale=1.0,
                            )
                        nc.vector.tensor_mul(
                            kt[:, co : co + cw], kr[:, 0:cw], kr[:, 0:cw]
                        )
                    kt_tiles[(hc, mh)] = kt
            # ---- mm2 phase: two C-halves, ys[block] one psum bank each ---
            for half in range(2):
                ys_b = {}
                for hc in range(NHC):
                    w2t = w2p.tile([TILE, NMH * 512], BF16, tag="w2",
                                   name=f"w2_{s}_{hc}_{half}")
                    nc.sync.dma_start(w2t[:], w2r[s, hc, half])
                    if hc % 3 == 2:
                        stage_a_hook()
                    for mh in range(NMH):
                        kt = kt_tiles[(hc, mh)]
                        for (c0, bw, gt) in slot_blocks[s]:
                            if (hc, mh) == (0, 0):
                                ys_b[c0] = ysp.tile([TILE, 512], F32, tag="ys",
                                                    name=f"ys{s}_{half}_{c0}")
                            nc.tensor.matmul(
                                ys_b[c0][0:bw, :],
                                kt[:, c0 : c0 + bw],
                                w2t[:, mh * 512 : (mh + 1) * 512],
                                start=(hc == 0 and mh == 0),
                                stop=(hc == NHC - 1 and mh == NMH - 1),
                            )
                # combine + output DMA per block half
                for (c0, bw, gt) in slot_blocks[s]:
                    ot = op.tile([TILE, 512], F32, tag="o")
                    nc.vector.scalar_tensor_tensor(
                        ot[0:bw, :],
                        ys_b[c0][0:bw, :],
                        sc_t[gt][0:bw],
                        x_t[gt][0:bw, half * 512 : (half + 1) * 512],
                        op0=OP.mult,
                        op1=OP.add,
                    )
                    r0 = soff + c0
                    nc.sync.dma_start(
                        yc[r0 : r0 + bw, half * 512 : (half + 1) * 512],
                        ot[0:bw, :],
                    )

    _split_excess_waits(nc, 1)
    return nc


# ---------------------------------------------------------------------------
# Host-side dispatch
# ---------------------------------------------------------------------------


MAXQ = 10  # max slot size in quanta (<=5 mm2 blocks -> 5 ys PSUM banks)


def _even_partitions(total, odd_last, min_part, max_len):
    """Yield tuples of slot sizes (quanta): all even except (iff odd_last)
    the final element which is odd; each >= min_part; ascending evens first."""
    def parts_even(rem, max_first, length):
        if rem == 0:
            yield ()
            return
        if length == 0:
            return
        f = min(rem, max_first, MAXQ)
        if f % 2 == 1:
            f -= 1
        while f >= min_part:
            for rest in parts_even(rem - f, f, length - 1):
                yield (f,) + rest
            f -= 2

    if odd_last:
        for last in range(min_part | 1, min(total, MAXQ) + 1, 2):
            rem = total - last
            if rem == 0:
                yield (last,)
                continue
            for ev in parts_even(rem, rem, max_len - 1):
                yield tuple(sorted(ev)) + (last,)
    else:
        for ev in parts_even(total, total, max_len):
            yield tuple(sorted(ev))


def _try_pack_q(qe, Tvec):
    """Greedy assign experts (quanta counts qe) to slot instances
    (8 per slot size). Returns assign[i] = [(slot_j, core_c), ...] or None."""
    avail = [list(range(NCORES)) for _ in Tvec]
    order_i = sorted(range(len(qe)), key=lambda i: -qe[i])
    assign = [None] * len(qe)
    sizes = sorted(range(len(Tvec)), key=lambda j: -Tvec[j])
    for i in order_i:
        rem = qe[i]
        inst = []
        while rem > 0:
            pick = None
            for j in sizes:
                if avail[j] and Tvec[j] <= rem:
                    pick = j
                    break
            if pick is None:
                for j in reversed(sizes):
                    if avail[j]:
                        pick = j
                        break
            if pick is None:
                return None
            c = avail[pick].pop(0)
            inst.append((pick, c))
            rem -= Tvec[pick]
        assign[i] = inst
    return assign


def _pack_slots_q(qe):
    """Choose per-core slot sizes (quanta, 128-aligned boundaries: evens
    then possibly one odd last) and expert->instance assignment."""
    total = sum(qe)
    Q0 = int(math.ceil(total / NCORES))
    for Q in range(Q0, Q0 + 6):
        odd_last = (Q % 2 == 1)
        cands = sorted(
            set(_even_partitions(Q, odd_last, 3 if odd_last else 2, 5)),
            key=lambda tv: (len(tv), -min(tv)),
        )
        # prefer all parts >= 4 quanta (mm1 stays moving-bound)
        for min4 in (True, False):
            for Tvec in cands:
                if min4 and min(Tvec) < 4:
                    continue
                a = _try_pack_q(list(qe), list(Tvec))
                if a is not None:
                    return list(Tvec), a
    raise RuntimeError(f"no packing found for quanta {qe}")


def _prepare(x, winners, gamma, beta, w1, w2, wc, bc):
    x = np.ascontiguousarray(np.asarray(x, dtype=np.float32))
    winners = np.asarray(winners).reshape(-1).astype(np.int64)
    gamma = np.asarray(gamma, dtype=np.float32)
    beta = np.asarray(beta, dtype=np.float32)
    w1 = np.asarray(w1, dtype=np.float32)
    w2 = np.asarray(w2, dtype=np.float32)
    wc = np.asarray(wc, dtype=np.float32)
    bc = np.asarray(bc, dtype=np.float32)

    B, T_, C = x.shape
    E, _, H = w1.shape
    N = B * T_
    xf = x.reshape(N, C)
    NKC = C // TILE
    HCHUNK = H // 8
    NMH = HCHUNK // TILE
    NHC = H // HCHUNK

    order = np.argsort(winners, kind="stable")
    counts = np.bincount(winners, minlength=E)
    present = [e for e in range(E) if counts[e] > 0]
    qe = [int(math.ceil(counts[e] / QT)) for e in present]

    Tvec, assign = _pack_slots_q(qe)
    S = len(Tvec)
    M = sum(Tvec) * QT

    slot_expert = [[present[0]] * S for _ in range(NCORES)]
    slot_idx = [
        [np.full(Tvec[j] * QT, -1, dtype=np.int64) for j in range(S)]
        for c in range(NCORES)
    ]
    pos = 0
    for i, e in enumerate(present):
        n_e = int(counts[e])
        toks = order[pos : pos + n_e]
        pos += n_e
        filled = 0
        for (j, c) in assign[i]:
            slot_expert[c][j] = e
            cap = Tvec[j] * QT
            take = min(cap, n_e - filled)
            if take > 0:
                slot_idx[c][j][:take] = toks[filled : filled + take]
                filled += take
        assert filled == n_e
    per_core_idx = [np.concatenate(slot_idx[c]) for c in range(NCORES)]

    zero_bias = bool(np.all(beta == 0.0))
    w1r_e, w2r_e, wcb_e, bc_e, b1b_e = {}, {}, {}, {}, {}
    for e in present:
        w1f = (w1[e] * gamma[:, None]).astype(NP_BF16)
        w2f = w2[e].astype(NP_BF16)
        # w1 chunk layout: [NHC, TILE(c within kc), NKC*HCHUNK]
        w1r_e[e] = np.ascontiguousarray(
            w1f.reshape(NKC, TILE, NHC, HCHUNK)
            .transpose(2, 1, 0, 3)
            .reshape(NHC, TILE, NKC * HCHUNK)
        )
        # w2 half-chunk layout: [NHC, 2(half), TILE(h within mh), NMH*512]
        w2r_e[e] = np.ascontiguousarray(
            w2f.reshape(NHC, NMH, TILE, 2, 512)
            .transpose(0, 3, 2, 1, 4)
            .reshape(NHC, 2, TILE, NMH * 512)
        )
        wcf = (wc[e] * gamma).astype(NP_BF16)
        wcb_e[e] = np.ascontiguousarray(np.broadcast_to(wcf[None, :], (TILE, C)))
        bc_e[e] = np.full((TILE, 1), float(bc[e] + float(beta @ wc[e])),
                          dtype=np.float32)
        if not zero_bias:
            b1 = beta @ w1[e]
            b1b_e[e] = np.ascontiguousarray(
                b1.reshape(H // TILE, TILE).T
            ).astype(np.float32)

    in_maps = []
    for c in range(NCORES):
        idx = per_core_idx[c]
        xcrows = np.zeros((M, C), dtype=np.float32)
        valid = idx >= 0
        xcrows[valid] = xf[idx[valid]]
        sl = slot_expert[c]
        m = {
            "xc": xcrows,
            "w1r": np.stack([w1r_e[e] for e in sl]),
            "w2r": np.stack([w2r_e[e] for e in sl]),
            "wcb": np.stack([wcb_e[e] for e in sl]),
            "bcs": np.stack([bc_e[e] for e in sl]),
        }
        if not zero_bias:
            m["b1b"] = np.stack([b1b_e[e] for e in sl])
        in_maps.append(m)

    meta = dict(
        B=B, T=T_, C=C, H=H, N=N, M=M, Tvec=Tvec, zero_bias=zero_bias,
        per_core_idx=per_core_idx,
    )
    return in_maps, meta


def _assemble(results, meta):
    N, C = meta["N"], meta["C"]
    out = np.empty((N, C), dtype=np.float32)
    seen = np.zeros(N, dtype=bool)
    for c in range(NCORES):
        idx = meta["per_core_idx"][c]
        valid = idx >= 0
        out[idx[valid]] = results[c]["yc"][valid]
        seen[idx[valid]] = True
    assert seen.all()
    return out.reshape(meta["B"], meta["T"], C)


def kernel_with_results(x, winners, gamma, beta, w1, w2, wc, bc, **run_kwargs):
    in_maps, meta = _prepare(x, winners, gamma, beta, w1, w2, wc, bc)
    nc = _build_program(
        meta["C"], meta["H"], meta["M"], meta["Tvec"], meta["zero_bias"]
    )
    res = run_bass_kernel_spmd(nc, in_maps, core_ids=list(range(NCORES)), **run_kwargs)
    return _assemble(res.results, meta), res


def kernel(x, winners, gamma, beta, w1, w2, wc, bc):
    out, _ = kernel_with_results(x, winners, gamma, beta, w1, w2, wc, bc)
    return out
